# revision 10
# baseline (speedup 1.0000x reference)
"""Trainium2 Bass kernel for gnn_message_passing (nn_CMP_67181878444960).

Strategy (8-core SPMD, no collectives):
  - Host converts the edge list into two dense [V, V] count matrices
    (pos / neg).  pooled = A @ feats is then a dense matmul: each core
    computes the pooled features for its 128 nodes by streaming the full
    feats matrix [1024, 16384] through the PE (K-tiled by 128), keeping
    pooled resident in SBUF.
  - The conv encoder is embarrassingly parallel over nodes.  Convs are 9
    shift-tap matmuls (contraction over channels on partitions).  To beat
    the 48-channel underutilization of the 128x128 PE array, each conv
    matmul round runs FOUR nodes concurrently as 4 independent 64x64 PE
    tiles (tile_position row/col halves); the final 48->16 conv runs
    EIGHT nodes concurrently as 8 independent 64x32 PE tiles.
  - x/h tiles are dense [128, 1024] (no spatial zero-padding): the
    identity tap (0,0) runs first with start=True covering the full
    window, every shifted tap then writes a clipped output window
    (PSUM's per-element write-then-accumulate handles the rest).
"""

import functools
import sys

import numpy as np

for _p in ("/opt/trn_rl_repo",):
    if _p not in sys.path:
        sys.path.insert(0, _p)

import concourse.tile as tile  # noqa: E402
from concourse import bacc, bass_utils, mybir  # noqa: E402
from concourse.tile_rust import add_dep_helper  # noqa: E402

F32 = mybir.dt.float32
BF16 = mybir.dt.bfloat16
AF = mybir.ActivationFunctionType

V, C, H = 1024, 16, 32
SP = H * H            # 1024 spatial
CHW = C * SP          # 16384
C3 = 3 * C            # 48 conv channels
NCORES = 8
NPC = V // NCORES     # 128 nodes per core
EPS = 1e-5

# identity tap FIRST: it writes the full output window with start=True so
# every PSUM element is write-initialized before clipped taps accumulate.
TAPS = [(0, 0), (-1, -1), (-1, 0), (-1, 1), (0, -1),
        (0, 1), (1, -1), (1, 0), (1, 1)]

# weight-column layout: 4 res layers (9 taps x 64 cols) + final (9 x 32)
_RES_W = 9 * 64
_TAP_OFF = [0, _RES_W, 2 * _RES_W, 3 * _RES_W, 4 * _RES_W]
WCOLS = 4 * _RES_W + 9 * 32


def _mi(inst):
    return getattr(inst, "ins", inst)


def _clip(r0, dy, dx):
    """Output-window clip for a 3x3 SAME tap on dense HxH planes."""
    rl = max(r0, -dy)
    rh = min(r0 + 16, H - dy)
    cl = max(0, -dx)
    ch = min(H, H - dx)
    return rl, rh, cl, ch


class _SlotGuard:
    """Explicitly order each pool slot's new first-writer after the previous
    occupant's last accessor (belt-and-braces against mis-synced reuse).

    begin() calls must follow allocation order per tag, end() calls too;
    begin #n pairs with end #n (separate counters allow several same-tag
    tiles in flight)."""

    def __init__(self):
        self.state = {}

    def begin(self, tag, bufs, writer_insts):
        st = self.state.setdefault(tag, [0, 0, {}])
        prev = st[2].get(st[0] % bufs)
        if prev is not None:
            for w in writer_insts:
                add_dep_helper(_mi(w), _mi(prev), True, "slot-reuse guard")
        st[0] += 1

    def end(self, tag, bufs, last_inst):
        st = self.state.setdefault(tag, [0, 0, {}])
        st[2][st[1] % bufs] = last_inst
        st[1] += 1


def build_kernel(tc, aps, npc, v):
    """Emit the per-core program. aps: dict of dram APs."""
    nc = tc.nc
    kt = v // 128            # K-tiles for pooling
    n_chunk = 512            # pooling column chunk
    nchunks = CHW // n_chunk

    feats_pool = aps["feats_pool"]
    feats_shard = aps["feats_shard"]
    a_lhsT = aps["a_lhsT"]
    wconv = aps["wconv"]
    biases = aps["biases"]
    out = aps["out"]

    guard = _SlotGuard()
    ctx = {"guard": guard}
    build_kernel._ctx = ctx

    with (
        tc.tile_pool(name="persist", bufs=1) as persist,
        tc.tile_pool(name="psum", bufs=8, space="PSUM") as psum_pool,
    ):
        # ---- persistent SBUF state ----
        wsb = persist.tile([128, WCOLS], BF16, tag="wsb")
        pooled = persist.tile([128, 2 * CHW], BF16, tag="pooled")
        bias_sb = persist.tile([128, 6], F32, tag="bias_sb")
        ctx["wsb"] = wsb
        ctx["bias_sb"] = bias_sb

        nc.sync.dma_start(wsb[:], wconv[:, :])
        nc.sync.dma_start(bias_sb[:], biases[:, :])

        # ================= stage 1: pooling =================
        with (
            tc.tile_pool(name="asb", bufs=1) as asb_pool,
            tc.tile_pool(name="fstage", bufs=4) as fstage,
        ):
            a_sb = asb_pool.tile([128, kt * 2 * npc], BF16)
            nc.sync.dma_start(a_sb[:], a_lhsT[:, :])
            for cc in range(nchunks):
                fs = fstage.tile([128, kt * n_chunk], BF16, tag="fs")
                d = nc.sync.dma_start(
                    fs[:], feats_pool[cc * 128:(cc + 1) * 128, :])
                guard.begin("fs", 4, [d])
                last_mm = None
                fs_r = fs[:].rearrange("p (k n) -> p k n", k=kt)
                a_r = a_sb[:].rearrange("p (k m) -> p k m", k=kt)
                for m in range(2):
                    pp = psum_pool.tile([128, n_chunk], F32, tag="ps")
                    for k in range(kt):
                        last_mm = nc.tensor.matmul(
                            pp[:npc, :],
                            a_r[:, k, m * npc:(m + 1) * npc],
                            fs_r[:, k, :],
                            start=(k == 0),
                            stop=(k == kt - 1),
                        )
                    nc.vector.tensor_copy(
                        pooled[:npc, m * CHW + cc * n_chunk:
                               m * CHW + (cc + 1) * n_chunk],
                        pp[:npc, :],
                    )
                guard.end("fs", 4, last_mm)

        # ================= stage 2: conv encoder =================
        ctx["pooled"] = pooled
        ctx["feats_shard"] = feats_shard
        with (
            tc.tile_pool(name="xt", bufs=6) as xpool,
            tc.tile_pool(name="ht", bufs=4) as hpool,
            tc.tile_pool(name="tmp", bufs=4) as tmppool,
            tc.tile_pool(name="ot", bufs=4) as opool,
            tc.tile_pool(name="nrm", bufs=4) as nrm,
        ):
            ctx["xpool"] = xpool
            ctx["hpool"] = hpool
            ctx["tmppool"] = tmppool
            for m in range(npc // 8):      # super-groups of 8 nodes
                xts = []
                for g2 in range(2):
                    g = 2 * m + g2
                    x1 = _assemble(tc, 4 * g + 0, 4 * g + 1)
                    x2 = _assemble(tc, 4 * g + 2, 4 * g + 3)
                    guard = ctx["guard"]
                    h1, h2, _, _ = _conv4(tc, psum_pool, 0, x1, x2,
                                          "relu", 0)
                    _, _, l1, l2 = _conv4(tc, psum_pool, 1, h1, h2,
                                          "resid", 4, dst=(x1, x2))
                    guard.end("h", 4, l1)
                    guard.end("h", 4, l2)
                    h1, h2, _, _ = _conv4(tc, psum_pool, 2, x1, x2,
                                          "relu", 1)
                    _, _, l1, l2 = _conv4(tc, psum_pool, 3, h1, h2,
                                          "resid", 5, dst=(x1, x2))
                    guard.end("h", 4, l1)
                    guard.end("h", 4, l2)
                    xts += [x1, x2]
                # ---- final conv (8 nodes via 64x32 tiles) + norm ----
                ot = []
                for b in range(2):
                    ot.append(opool.tile([128, SP], F32, tag="ot", name=f"ot{b}"))
                ot_writers = {0: [], 1: []}
                f_last = {}
                for ht in range(2):
                    r0 = 16 * ht
                    fb = [psum_pool.tile([128, 512], F32, tag="ps", name=f"fb{_b}")
                          for _b in range(2)]
                    for i, (dy, dx) in enumerate(TAPS):
                        woff = _TAP_OFF[4] + i * 32
                        rl, rh, cl, ch = _clip(r0, dy, dx)
                        for j in range(4):
                            xv = xts[j][:].rearrange(
                                "p (r c) -> p r c", c=H)
                            for row, base in ((0, 0), (1, 64)):
                                pv = fb[row][:].rearrange(
                                    "p (r c) -> p r c", c=H)
                                mm = nc.tensor.matmul(
                                    pv[32 * j:32 * j + 32,
                                       rl - r0:rh - r0, cl:ch],
                                    wsb[base:base + 48, woff:woff + 32],
                                    xv[base:base + 48,
                                       rl + dy:rh + dy, cl + dx:ch + dx],
                                    start=(i == 0), stop=(i == 8),
                                    skip_group_check=True,
                                    tile_position=(base, 32 * j),
                                )
                                f_last[(ht, row, j)] = mm
                    for row in range(2):
                        ap_i = nc.scalar.activation(
                            ot[row][:, r0 * H:(r0 + 16) * H], fb[row][:],
                            AF.Identity, bias=bias_sb[:, 2:3],
                        )
                        ot_writers[row].append(ap_i)
                for row in range(2):
                    ctx["guard"].begin("ot", 4, ot_writers[row])
                # x tiles fully consumed by final conv
                for j in range(4):
                    ctx["guard"].end("x", 6, f_last[(1, 1, j)])
                # instance norm + relu + out DMA per bank
                for row in range(2):
                    stats = nrm.tile([128, 12], F32, tag="stats")
                    mv = nrm.tile([128, 2], F32, tag="mv")
                    sc = nrm.tile([128, 3], F32, tag="sc")
                    nc.vector.bn_stats(stats[:, 0:6], ot[row][:, 0:512])
                    nc.vector.bn_stats(stats[:, 6:12], ot[row][:, 512:1024])
                    nc.vector.bn_aggr(mv[:], stats[:])
                    nc.scalar.activation(sc[:, 0:1], mv[:, 1:2], AF.Sqrt,
                                         bias=bias_sb[:, 3:4])
                    nc.vector.reciprocal(sc[:, 1:2], sc[:, 0:1])
                    nc.vector.tensor_scalar(
                        sc[:, 2:3], mv[:, 0:1], sc[:, 1:2], -1.0,
                        op0=mybir.AluOpType.mult, op1=mybir.AluOpType.mult,
                    )
                    fin = opool.tile([128, SP], F32, tag="fin")
                    ap_f = nc.scalar.activation(
                        fin[:], ot[row][:], AF.Relu,
                        bias=sc[:, 2:3], scale=sc[:, 1:2],
                    )
                    ctx["guard"].begin("fin", 4, [ap_f])
                    ctx["guard"].end("ot", 4, ap_f)
                    last_d = None
                    for j in range(4):
                        node = 8 * m + 2 * j + row
                        last_d = nc.sync.dma_start(
                            out[node:node + 1, :].rearrange(
                                "o (c s) -> (o c) s", c=C),
                            fin[32 * j:32 * j + 16, :],
                        )
                    ctx["guard"].end("fin", 4, last_d)


def _assemble(tc, n0, n1):
    """Build the [n0; n1] x tile: channels 0:48 node n0, 64:112 node n1."""
    nc = tc.nc
    ctx = build_kernel._ctx
    x = ctx["xpool"].tile([128, SP], BF16, tag="x")
    # engine APs need 32-aligned partition bases; the pooled DMA below
    # overwrites 32:48 after this zeroes the 48:64 junk band.
    wrts = [nc.vector.memset(x[32:64, :], 0.0)]
    for base, n in ((0, n0), (64, n1)):
        wrts.append(nc.sync.dma_start(
            x[base:base + 16, :],
            ctx["feats_shard"][n:n + 1, :].rearrange(
                "o (c s) -> (o c) s", c=C),
        ))
        for mm in range(2):
            wrts.append(nc.gpsimd.dma_start(
                x[base + 16 * (mm + 1):base + 16 * (mm + 2), :],
                ctx["pooled"][n:n + 1, mm * CHW:(mm + 1) * CHW],
            ))
    ctx["guard"].begin("x", 6, wrts)
    return x


def _conv4(tc, psum_pool, layer, in1, in2, mode, bias_col, dst=None):
    """One 3x3 'SAME' conv over FOUR nodes via 4 concurrent 64x64 PE tiles.

    in1 = [A; B], in2 = [C; D] (node channels at partitions 0:48, 64:112).
    Bank pA collects [top(in1); top(in2)], pB collects [bot(in1); bot(in2)].
    mode 'relu': returns h tiles (h1 = ACT(pA), h2 = ACT(pB)).
    mode 'resid': dst = (d1, d2); d1 += pA + bias, d2 += pB + bias.
    """
    nc = tc.nc
    ctx = build_kernel._ctx
    wsb, bias_sb, guard = ctx["wsb"], ctx["bias_sb"], ctx["guard"]

    outs = [None, None]
    if mode == "relu":
        outs = [ctx["hpool"].tile([128, SP], BF16, tag="h", name="h1"),
                ctx["hpool"].tile([128, SP], BF16, tag="h", name="h2")]

    h_writers = {0: [], 1: []}
    last_in1 = last_in2 = None
    for ht in range(2):
        r0 = 16 * ht
        pA = psum_pool.tile([128, 512], F32, tag="ps")
        pB = psum_pool.tile([128, 512], F32, tag="ps")
        pAv = pA[:].rearrange("p (r c) -> p r c", c=H)
        pBv = pB[:].rearrange("p (r c) -> p r c", c=H)
        i1v = in1[:].rearrange("p (r c) -> p r c", c=H)
        i2v = in2[:].rearrange("p (r c) -> p r c", c=H)
        for i, (dy, dx) in enumerate(TAPS):
            woff = _TAP_OFF[layer] + i * 64
            w_top = wsb[0:48, woff:woff + 64]
            w_bot = wsb[64:112, woff:woff + 64]
            rl, rh, cl, ch = _clip(r0, dy, dx)
            orow = slice(rl - r0, rh - r0)
            ocol = slice(cl, ch)
            irow = slice(rl + dy, rh + dy)
            icol = slice(cl + dx, ch + dx)
            st_, sp_ = (i == 0), (i == 8)
            # pA = [top(in1); top(in2)], pB = [bot(in1); bot(in2)]:
            # tile (64,0) writes psum 0:64 so in1's bottom goes there.
            nc.tensor.matmul(
                pAv[0:64, orow, ocol], w_top, i1v[0:48, irow, icol],
                start=st_, stop=sp_, skip_group_check=True)
            last_in1 = nc.tensor.matmul(
                pBv[0:64, orow, ocol], w_bot, i1v[64:112, irow, icol],
                start=st_, stop=sp_, skip_group_check=True)
            nc.tensor.matmul(
                pAv[64:128, orow, ocol], w_top, i2v[0:48, irow, icol],
                start=st_, stop=sp_, skip_group_check=True)
            last_in2 = nc.tensor.matmul(
                pBv[64:128, orow, ocol], w_bot, i2v[64:112, irow, icol],
                start=st_, stop=sp_, skip_group_check=True)
        cols = slice(r0 * H, (r0 + 16) * H)
        for bank, p in ((0, pA), (1, pB)):
            if mode == "relu":
                t = nc.scalar.activation(
                    outs[bank][0:112, cols], p[0:112, :],
                    AF.Relu, bias=bias_sb[0:112, bias_col:bias_col + 1],
                )
                h_writers[bank].append(t)
            else:
                tmp = ctx["tmppool"].tile([128, 512], BF16, tag="tmp")
                ap_i = nc.scalar.activation(
                    tmp[0:112, :], p[0:112, :],
                    AF.Identity, bias=bias_sb[0:112, bias_col:bias_col + 1],
                )
                guard.begin("tmp", 4, [ap_i])
                t = nc.vector.tensor_add(
                    dst[bank][0:112, cols],
                    dst[bank][0:112, cols],
                    tmp[0:112, :],
                )
                guard.end("tmp", 4, t)

    if mode == "relu":
        guard.begin("h", 4, h_writers[0])
        guard.begin("h", 4, h_writers[1])
    return outs[0], outs[1], last_in1, last_in2


# ======================= host side =======================

def _prep_weights(w_list, b_list):
    """Pack conv weights into the [128, WCOLS] bf16 lhsT array.

    Res layers: per tap a [48 -> 64] column block (16 zero cols pad M to
    64); final layer: [48 -> 32] blocks.  Weight rows live at partitions
    0:48 AND (duplicated) 64:112 so both PE row-halves can load them.
    """
    wsb = np.zeros((128, WCOLS), np.float32)
    for layer, w in enumerate(w_list):
        co = w.shape[0]
        blk = 64 if layer < 4 else 32
        for i, (dy, dx) in enumerate(TAPS):
            ky, kx = dy + 1, dx + 1
            lt = np.ascontiguousarray(w[:, :, ky, kx].T)  # [C_in, C_out]
            off = _TAP_OFF[layer] + i * blk
            wsb[0:48, off:off + co] = lt
            wsb[64:112, off:off + co] = lt
    import ml_dtypes
    return wsb.astype(ml_dtypes.bfloat16)


def _prep_biases(b1a, b2a, bf, b1b, b2b):
    bias = np.zeros((128, 6), np.float32)
    for base in (0, 64):
        bias[base:base + 48, 0] = b1a
        bias[base:base + 48, 1] = b2a
        bias[base:base + 48, 4] = b1b
        bias[base:base + 48, 5] = b2b
    for strip in range(4):
        bias[32 * strip:32 * strip + 16, 2] = bf
    bias[:, 3] = EPS
    return bias


def _build_adjacency(edges, v):
    src, lab, dst = edges[:, 0], edges[:, 1], edges[:, 2]
    a = np.zeros((2, v, v), np.float32)
    for mi, mask in enumerate((lab > 0, lab < 0)):
        s, d = src[mask], dst[mask]
        np.add.at(a[mi], (d, s), 1.0)
        np.add.at(a[mi], (s, d), 1.0)
    return a


@functools.lru_cache(maxsize=2)
def _build_module(npc, v, ncores):
    nc = bacc.Bacc(
        "TRN2", target_bir_lowering=False, debug=False,
        enable_asserts=False, num_devices=ncores,
    )
    aps = {
        "feats_pool": nc.dram_tensor("feats_pool", [(CHW // 512) * 128,
                                     (v // 128) * 512], BF16,
                                     kind="ExternalInput").ap(),
        "feats_shard": nc.dram_tensor("feats_shard", [npc, CHW], BF16,
                                      kind="ExternalInput").ap(),
        "a_lhsT": nc.dram_tensor("a_lhsT", [128, (v // 128) * 2 * npc], BF16,
                                 kind="ExternalInput").ap(),
        "wconv": nc.dram_tensor("wconv", [128, WCOLS], BF16,
                                kind="ExternalInput").ap(),
        "biases": nc.dram_tensor("biases", [128, 6], F32,
                                 kind="ExternalInput").ap(),
        "out": nc.dram_tensor("out", [npc, CHW], F32,
                              kind="ExternalOutput").ap(),
    }
    with tile.TileContext(nc) as tc:
        build_kernel(tc, aps, npc, v)
    nc.compile()
    return nc


def make_in_maps(feats, edges, w1a, b1a, w1b, b1b, w2a, b2a, w2b, b2b,
                 wf, bf, ncores=NCORES, v=V):
    feats = np.ascontiguousarray(np.asarray(feats, np.float32)).reshape(v, CHW)
    edges = np.asarray(edges)
    npc = v // ncores
    a = _build_adjacency(edges, v)
    wsb = _prep_weights(
        [np.asarray(w) for w in (w1a, w1b, w2a, w2b, wf)],
        [np.asarray(b) for b in (b1a, b1b, b2a, b2b, bf)],
    )
    bias = _prep_biases(np.asarray(b1a), np.asarray(b2a), np.asarray(bf),
                        np.asarray(b1b), np.asarray(b2b))
    in_maps = []
    for i in range(ncores):
        rows = slice(i * npc, (i + 1) * npc)
        a_sel = np.concatenate([a[0, rows], a[1, rows]], axis=0)  # [2*npc, V]
        import ml_dtypes
        kt = v // 128
        nch = CHW // 512
        fp = feats.reshape(kt, 128, nch, 512).transpose(2, 1, 0, 3)
        fp = np.ascontiguousarray(fp).reshape(nch * 128, kt * 512)
        alt = a_sel.T.reshape(kt, 128, 2 * npc).transpose(1, 0, 2)
        alt = np.ascontiguousarray(alt).reshape(128, kt * 2 * npc)
        in_maps.append({
            "feats_pool": fp.astype(ml_dtypes.bfloat16),
            "feats_shard": np.ascontiguousarray(feats[rows]).astype(
                ml_dtypes.bfloat16),
            "a_lhsT": alt.astype(ml_dtypes.bfloat16),
            "wconv": wsb,
            "biases": bias,
        })
    return in_maps


def run(inputs, trace=False):
    in_maps = make_in_maps(**inputs)
    nc = _build_module(NPC, V, NCORES)
    res = bass_utils.run_bass_kernel_spmd(
        nc, in_maps, core_ids=list(range(NCORES)), trace=trace,
    )
    out = np.concatenate(
        [res.results[i]["out"] for i in range(NCORES)], axis=0
    ).reshape(V, C, H, H)
    return out, res


def kernel(**inputs):
    out, _ = run(inputs, trace=False)
    return out


# revision 16
# speedup vs baseline: 1.0270x; 1.0270x over previous
"""Trainium2 Bass kernel for gnn_message_passing (nn_CMP_67181878444960).

Strategy (8-core SPMD, no collectives):
  - Host converts the edge list into two dense [V, V] count matrices
    (pos / neg).  pooled = A @ feats is then a dense matmul: each core
    computes the pooled features for its 128 nodes by streaming the full
    feats matrix [1024, 16384] through the PE (K-tiled by 128), keeping
    pooled resident in SBUF.
  - The conv encoder is embarrassingly parallel over nodes.  Convs are 9
    shift-tap matmuls (contraction over channels on partitions).  To beat
    the 48-channel underutilization of the 128x128 PE array, each conv
    matmul round runs FOUR nodes concurrently as 4 independent 64x64 PE
    tiles (tile_position row/col halves); the final 48->16 conv runs
    EIGHT nodes concurrently as 8 independent 64x32 PE tiles.
  - x/h tiles are dense [128, 1024] (no spatial zero-padding): the
    identity tap (0,0) runs first with start=True covering the full
    window, every shifted tap then writes a clipped output window
    (PSUM's per-element write-then-accumulate handles the rest).
"""

import functools
import sys

import numpy as np

for _p in ("/opt/trn_rl_repo",):
    if _p not in sys.path:
        sys.path.insert(0, _p)

import concourse.tile as tile  # noqa: E402
from concourse import bacc, bass_utils, mybir  # noqa: E402
from concourse.tile_rust import add_dep_helper  # noqa: E402

F32 = mybir.dt.float32
BF16 = mybir.dt.bfloat16
AF = mybir.ActivationFunctionType

V, C, H = 1024, 16, 32
SP = H * H            # 1024 spatial
CHW = C * SP          # 16384
C3 = 3 * C            # 48 conv channels
NCORES = 8
NPC = V // NCORES     # 128 nodes per core
EPS = 1e-5

# identity tap FIRST: it writes the full output window with start=True so
# every PSUM element is write-initialized before clipped taps accumulate.
TAPS = [(0, 0), (-1, -1), (-1, 0), (-1, 1), (0, -1),
        (0, 1), (1, -1), (1, 0), (1, 1)]

# weight-column layout: 4 res layers (9 taps x 64 cols) + final (9 x 32)
_RES_W = 9 * 64
_TAP_OFF = [0, _RES_W, 2 * _RES_W, 3 * _RES_W, 4 * _RES_W]
WCOLS = 4 * _RES_W + 9 * 32


def _mi(inst):
    return getattr(inst, "ins", inst)


def _clip(r0, dy, dx):
    """Output-window clip for a 3x3 SAME tap on dense HxH planes."""
    rl = max(r0, -dy)
    rh = min(r0 + 16, H - dy)
    cl = max(0, -dx)
    ch = min(H, H - dx)
    return rl, rh, cl, ch


class _SlotGuard:
    """Explicitly order each pool slot's new first-writer after the previous
    occupant's last accessor (belt-and-braces against mis-synced reuse).

    begin() calls must follow allocation order per tag, end() calls too;
    begin #n pairs with end #n (separate counters allow several same-tag
    tiles in flight)."""

    def __init__(self):
        self.state = {}

    def begin(self, tag, bufs, writer_insts):
        st = self.state.setdefault(tag, [0, 0, {}])
        prev = st[2].get(st[0] % bufs)
        if prev is not None:
            for w in writer_insts:
                add_dep_helper(_mi(w), _mi(prev), True, "slot-reuse guard")
        st[0] += 1

    def end(self, tag, bufs, last_inst):
        st = self.state.setdefault(tag, [0, 0, {}])
        st[2][st[1] % bufs] = last_inst
        st[1] += 1


def build_kernel(tc, aps, npc, v):
    """Emit the per-core program. aps: dict of dram APs."""
    nc = tc.nc
    kt = v // 128            # K-tiles for pooling
    n_chunk = 512            # pooling column chunk
    nchunks = CHW // n_chunk

    feats_pool = aps["feats_pool"]
    feats_shard = aps["feats_shard"]
    a_lhsT = aps["a_lhsT"]
    wconv = aps["wconv"]
    biases = aps["biases"]
    out = aps["out"]

    guard = _SlotGuard()
    ctx = {"guard": guard}
    build_kernel._ctx = ctx

    with (
        tc.tile_pool(name="persist", bufs=1) as persist,
        tc.tile_pool(name="psum", bufs=8, space="PSUM") as psum_pool,
    ):
        # ---- persistent SBUF state ----
        wsb = persist.tile([128, WCOLS], BF16, tag="wsb")
        pooled = persist.tile([128, 2 * CHW], BF16, tag="pooled")
        bias_sb = persist.tile([128, 6], F32, tag="bias_sb")
        ctx["wsb"] = wsb
        ctx["bias_sb"] = bias_sb

        nc.scalar.dma_start(wsb[:], wconv[:, :])
        nc.scalar.dma_start(bias_sb[:], biases[:, :])

        # ================= stage 1: pooling =================
        with (
            tc.tile_pool(name="asb", bufs=1) as asb_pool,
            tc.tile_pool(name="fstage", bufs=4) as fstage,
        ):
            a_sb = asb_pool.tile([128, kt * 2 * npc], BF16)
            nc.gpsimd.dma_start(a_sb[:], a_lhsT[:, :])
            for cc in range(nchunks):
                fs = fstage.tile([128, kt * n_chunk], BF16, tag="fs")
                d = nc.sync.dma_start(
                    fs[:], feats_pool[cc * 128:(cc + 1) * 128, :])
                guard.begin("fs", 4, [d])
                last_mm = None
                fs_r = fs[:].rearrange("p (k n) -> p k n", k=kt)
                a_r = a_sb[:].rearrange("p (k m) -> p k m", k=kt)
                for m in range(2):
                    pp = psum_pool.tile([128, n_chunk], F32, tag="ps")
                    for k in range(kt):
                        last_mm = nc.tensor.matmul(
                            pp[:npc, :],
                            a_r[:, k, m * npc:(m + 1) * npc],
                            fs_r[:, k, :],
                            start=(k == 0),
                            stop=(k == kt - 1),
                        )
                    nc.vector.tensor_copy(
                        pooled[:npc, m * CHW + cc * n_chunk:
                               m * CHW + (cc + 1) * n_chunk],
                        pp[:npc, :],
                    )
                guard.end("fs", 4, last_mm)

        # ================= stage 2: conv encoder =================
        ctx["pooled"] = pooled
        ctx["feats_shard"] = feats_shard
        with (
            tc.tile_pool(name="xt", bufs=8) as xpool,
            tc.tile_pool(name="ht", bufs=4) as hpool,
            tc.tile_pool(name="tmp", bufs=4) as tmppool,
            tc.tile_pool(name="ot", bufs=4) as opool,
            tc.tile_pool(name="nrm", bufs=4) as nrm,
        ):
            ctx["xpool"] = xpool
            ctx["hpool"] = hpool
            ctx["tmppool"] = tmppool
            nsg = npc // 8
            # assembly runs one super-group ahead of compute so its DMAs
            # overlap the previous group's convs (no PE bubble).
            xts_next = [_assemble(tc, 8 * 0 + 2 * t, 8 * 0 + 2 * t + 1)
                        for t in range(4)]
            for m in range(nsg):           # super-groups of 8 nodes
                xts_in = xts_next
                if m + 1 < nsg:
                    xts_next = [_assemble(tc, 8 * (m + 1) + 2 * t,
                                          8 * (m + 1) + 2 * t + 1)
                                for t in range(4)]
                xts = []
                for g2 in range(2):
                    x1, x2 = xts_in[2 * g2], xts_in[2 * g2 + 1]
                    guard = ctx["guard"]
                    h1, h2, _, _ = _conv4(tc, psum_pool, 0, x1, x2,
                                          "relu", 0)
                    _, _, l1, l2 = _conv4(tc, psum_pool, 1, h1, h2,
                                          "resid", 4, dst=(x1, x2))
                    guard.end("h", 4, l1)
                    guard.end("h", 4, l2)
                    h1, h2, _, _ = _conv4(tc, psum_pool, 2, x1, x2,
                                          "relu", 1)
                    _, _, l1, l2 = _conv4(tc, psum_pool, 3, h1, h2,
                                          "resid", 5, dst=(x1, x2))
                    guard.end("h", 4, l1)
                    guard.end("h", 4, l2)
                    xts += [x1, x2]
                # ---- final conv (8 nodes via 64x32 tiles) + norm ----
                ot = []
                for b in range(2):
                    ot.append(opool.tile([128, SP], F32, tag="ot", name=f"ot{b}"))
                ot_writers = {0: [], 1: []}
                f_last = {}
                for ht in range(2):
                    r0 = 16 * ht
                    fb = [psum_pool.tile([128, 512], F32, tag="ps", name=f"fb{_b}")
                          for _b in range(2)]
                    for i, (dy, dx) in enumerate(TAPS):
                        woff = _TAP_OFF[4] + i * 32
                        rl, rh, cl, ch = _clip(r0, dy, dx)
                        for j in range(4):
                            xv = xts[j][:].rearrange(
                                "p (r c) -> p r c", c=H)
                            for row, base in ((0, 0), (1, 64)):
                                pv = fb[row][:].rearrange(
                                    "p (r c) -> p r c", c=H)
                                mm = nc.tensor.matmul(
                                    pv[32 * j:32 * j + 32,
                                       rl - r0:rh - r0, cl:ch],
                                    wsb[base:base + 48, woff:woff + 32],
                                    xv[base:base + 48,
                                       rl + dy:rh + dy, cl + dx:ch + dx],
                                    start=(i == 0), stop=(i == 8),
                                    skip_group_check=True,
                                    tile_position=(base, 32 * j),
                                )
                                f_last[(ht, row, j)] = mm
                    for row in range(2):
                        ap_i = nc.scalar.activation(
                            ot[row][:, r0 * H:(r0 + 16) * H], fb[row][:],
                            AF.Identity, bias=bias_sb[:, 2:3],
                        )
                        ot_writers[row].append(ap_i)
                for row in range(2):
                    ctx["guard"].begin("ot", 4, ot_writers[row])
                # x tiles fully consumed by final conv
                for j in range(4):
                    ctx["guard"].end("x", 8, f_last[(1, 1, j)])
                # instance norm + relu + out DMA per bank
                for row in range(2):
                    stats = nrm.tile([128, 12], F32, tag="stats")
                    mv = nrm.tile([128, 2], F32, tag="mv")
                    sc = nrm.tile([128, 3], F32, tag="sc")
                    nc.vector.bn_stats(stats[:, 0:6], ot[row][:, 0:512])
                    nc.vector.bn_stats(stats[:, 6:12], ot[row][:, 512:1024])
                    nc.vector.bn_aggr(mv[:], stats[:])
                    nc.scalar.activation(sc[:, 0:1], mv[:, 1:2], AF.Sqrt,
                                         bias=bias_sb[:, 3:4])
                    nc.vector.reciprocal(sc[:, 1:2], sc[:, 0:1])
                    nc.vector.tensor_scalar(
                        sc[:, 2:3], mv[:, 0:1], sc[:, 1:2], -1.0,
                        op0=mybir.AluOpType.mult, op1=mybir.AluOpType.mult,
                    )
                    fin = opool.tile([128, SP], F32, tag="fin")
                    ap_f = nc.scalar.activation(
                        fin[:], ot[row][:], AF.Relu,
                        bias=sc[:, 2:3], scale=sc[:, 1:2],
                    )
                    ctx["guard"].begin("fin", 4, [ap_f])
                    ctx["guard"].end("ot", 4, ap_f)
                    last_d = None
                    for j in range(4):
                        node = 8 * m + 2 * j + row
                        last_d = nc.sync.dma_start(
                            out[node:node + 1, :].rearrange(
                                "o (c s) -> (o c) s", c=C),
                            fin[32 * j:32 * j + 16, :],
                        )
                    ctx["guard"].end("fin", 4, last_d)


def _assemble(tc, n0, n1):
    """Build the [n0; n1] x tile: channels 0:48 node n0, 64:112 node n1."""
    nc = tc.nc
    ctx = build_kernel._ctx
    x = ctx["xpool"].tile([128, SP], BF16, tag="x", name="x")
    wrts = []
    for base, n in ((0, n0), (64, n1)):
        wrts.append(nc.sync.dma_start(
            x[base:base + 16, :],
            ctx["feats_shard"][n:n + 1, :].rearrange(
                "o (c s) -> (o c) s", c=C),
        ))
        for mm in range(2):
            wrts.append(nc.gpsimd.dma_start(
                x[base + 16 * (mm + 1):base + 16 * (mm + 2), :],
                ctx["pooled"][n:n + 1, mm * CHW:(mm + 1) * CHW],
            ))
    ctx["guard"].begin("x", 8, wrts)
    return x


def _conv4(tc, psum_pool, layer, in1, in2, mode, bias_col, dst=None):
    """One 3x3 'SAME' conv over FOUR nodes via 4 concurrent 64x64 PE tiles.

    in1 = [A; B], in2 = [C; D] (node channels at partitions 0:48, 64:112).
    Bank pA collects [top(in1); top(in2)], pB collects [bot(in1); bot(in2)].
    mode 'relu': returns h tiles (h1 = ACT(pA), h2 = ACT(pB)).
    mode 'resid': dst = (d1, d2); d1 += pA + bias, d2 += pB + bias.
    """
    nc = tc.nc
    ctx = build_kernel._ctx
    wsb, bias_sb, guard = ctx["wsb"], ctx["bias_sb"], ctx["guard"]

    outs = [None, None]
    if mode == "relu":
        outs = [ctx["hpool"].tile([128, SP], BF16, tag="h", name="h1"),
                ctx["hpool"].tile([128, SP], BF16, tag="h", name="h2")]

    h_writers = {0: [], 1: []}
    last_in1 = last_in2 = None
    for ht in range(2):
        r0 = 16 * ht
        pA = psum_pool.tile([128, 512], F32, tag="ps")
        pB = psum_pool.tile([128, 512], F32, tag="ps")
        pAv = pA[:].rearrange("p (r c) -> p r c", c=H)
        pBv = pB[:].rearrange("p (r c) -> p r c", c=H)
        i1v = in1[:].rearrange("p (r c) -> p r c", c=H)
        i2v = in2[:].rearrange("p (r c) -> p r c", c=H)
        for i, (dy, dx) in enumerate(TAPS):
            woff = _TAP_OFF[layer] + i * 64
            w_top = wsb[0:48, woff:woff + 64]
            w_bot = wsb[64:112, woff:woff + 64]
            rl, rh, cl, ch = _clip(r0, dy, dx)
            orow = slice(rl - r0, rh - r0)
            ocol = slice(cl, ch)
            irow = slice(rl + dy, rh + dy)
            icol = slice(cl + dx, ch + dx)
            st_, sp_ = (i == 0), (i == 8)
            # pA = [top(in1); top(in2)], pB = [bot(in1); bot(in2)]:
            # tile (64,0) writes psum 0:64 so in1's bottom goes there.
            nc.tensor.matmul(
                pAv[0:64, orow, ocol], w_top, i1v[0:48, irow, icol],
                start=st_, stop=sp_, skip_group_check=True)
            last_in1 = nc.tensor.matmul(
                pBv[0:64, orow, ocol], w_bot, i1v[64:112, irow, icol],
                start=st_, stop=sp_, skip_group_check=True)
            nc.tensor.matmul(
                pAv[64:128, orow, ocol], w_top, i2v[0:48, irow, icol],
                start=st_, stop=sp_, skip_group_check=True)
            last_in2 = nc.tensor.matmul(
                pBv[64:128, orow, ocol], w_bot, i2v[64:112, irow, icol],
                start=st_, stop=sp_, skip_group_check=True)
        cols = slice(r0 * H, (r0 + 16) * H)
        for bank, p in ((0, pA), (1, pB)):
            if mode == "relu":
                t = nc.scalar.activation(
                    outs[bank][0:112, cols], p[0:112, :],
                    AF.Relu, bias=bias_sb[0:112, bias_col:bias_col + 1],
                )
                h_writers[bank].append(t)
            else:
                tmp = ctx["tmppool"].tile([128, 512], BF16, tag="tmp")
                ap_i = nc.scalar.activation(
                    tmp[0:112, :], p[0:112, :],
                    AF.Identity, bias=bias_sb[0:112, bias_col:bias_col + 1],
                )
                guard.begin("tmp", 4, [ap_i])
                # split halves so x's junk band 48:64 is never accessed
                # (it is never written: assembly is pure DMA now)
                nc.vector.tensor_add(
                    dst[bank][0:48, cols], dst[bank][0:48, cols],
                    tmp[0:48, :],
                )
                t = nc.vector.tensor_add(
                    dst[bank][64:112, cols], dst[bank][64:112, cols],
                    tmp[64:112, :],
                )
                guard.end("tmp", 4, t)

    if mode == "relu":
        guard.begin("h", 4, h_writers[0])
        guard.begin("h", 4, h_writers[1])
    return outs[0], outs[1], last_in1, last_in2


# ======================= host side =======================

def _prep_weights(w_list, b_list):
    """Pack conv weights into the [128, WCOLS] bf16 lhsT array.

    Res layers: per tap a [48 -> 64] column block (16 zero cols pad M to
    64); final layer: [48 -> 32] blocks.  Weight rows live at partitions
    0:48 AND (duplicated) 64:112 so both PE row-halves can load them.
    """
    wsb = np.zeros((128, WCOLS), np.float32)
    for layer, w in enumerate(w_list):
        co = w.shape[0]
        blk = 64 if layer < 4 else 32
        for i, (dy, dx) in enumerate(TAPS):
            ky, kx = dy + 1, dx + 1
            lt = np.ascontiguousarray(w[:, :, ky, kx].T)  # [C_in, C_out]
            off = _TAP_OFF[layer] + i * blk
            wsb[0:48, off:off + co] = lt
            wsb[64:112, off:off + co] = lt
    import ml_dtypes
    return wsb.astype(ml_dtypes.bfloat16)


def _prep_biases(b1a, b2a, bf, b1b, b2b):
    bias = np.zeros((128, 6), np.float32)
    for base in (0, 64):
        bias[base:base + 48, 0] = b1a
        bias[base:base + 48, 1] = b2a
        bias[base:base + 48, 4] = b1b
        bias[base:base + 48, 5] = b2b
    for strip in range(4):
        bias[32 * strip:32 * strip + 16, 2] = bf
    bias[:, 3] = EPS
    return bias


def _build_adjacency(edges, v):
    src, lab, dst = edges[:, 0], edges[:, 1], edges[:, 2]
    a = np.zeros((2, v, v), np.float32)
    for mi, mask in enumerate((lab > 0, lab < 0)):
        s, d = src[mask], dst[mask]
        np.add.at(a[mi], (d, s), 1.0)
        np.add.at(a[mi], (s, d), 1.0)
    return a


@functools.lru_cache(maxsize=2)
def _build_module(npc, v, ncores):
    nc = bacc.Bacc(
        "TRN2", target_bir_lowering=False, debug=False,
        enable_asserts=False, num_devices=ncores,
    )
    aps = {
        "feats_pool": nc.dram_tensor("feats_pool", [(CHW // 512) * 128,
                                     (v // 128) * 512], BF16,
                                     kind="ExternalInput").ap(),
        "feats_shard": nc.dram_tensor("feats_shard", [npc, CHW], BF16,
                                      kind="ExternalInput").ap(),
        "a_lhsT": nc.dram_tensor("a_lhsT", [128, (v // 128) * 2 * npc], BF16,
                                 kind="ExternalInput").ap(),
        "wconv": nc.dram_tensor("wconv", [128, WCOLS], BF16,
                                kind="ExternalInput").ap(),
        "biases": nc.dram_tensor("biases", [128, 6], F32,
                                 kind="ExternalInput").ap(),
        "out": nc.dram_tensor("out", [npc, CHW], F32,
                              kind="ExternalOutput").ap(),
    }
    with tile.TileContext(nc) as tc:
        build_kernel(tc, aps, npc, v)
    nc.compile()
    return nc


def make_in_maps(feats, edges, w1a, b1a, w1b, b1b, w2a, b2a, w2b, b2b,
                 wf, bf, ncores=NCORES, v=V):
    feats = np.ascontiguousarray(np.asarray(feats, np.float32)).reshape(v, CHW)
    edges = np.asarray(edges)
    npc = v // ncores
    a = _build_adjacency(edges, v)
    wsb = _prep_weights(
        [np.asarray(w) for w in (w1a, w1b, w2a, w2b, wf)],
        [np.asarray(b) for b in (b1a, b1b, b2a, b2b, bf)],
    )
    bias = _prep_biases(np.asarray(b1a), np.asarray(b2a), np.asarray(bf),
                        np.asarray(b1b), np.asarray(b2b))
    in_maps = []
    for i in range(ncores):
        rows = slice(i * npc, (i + 1) * npc)
        a_sel = np.concatenate([a[0, rows], a[1, rows]], axis=0)  # [2*npc, V]
        import ml_dtypes
        kt = v // 128
        nch = CHW // 512
        fp = feats.reshape(kt, 128, nch, 512).transpose(2, 1, 0, 3)
        fp = np.ascontiguousarray(fp).reshape(nch * 128, kt * 512)
        alt = a_sel.T.reshape(kt, 128, 2 * npc).transpose(1, 0, 2)
        alt = np.ascontiguousarray(alt).reshape(128, kt * 2 * npc)
        in_maps.append({
            "feats_pool": fp.astype(ml_dtypes.bfloat16),
            "feats_shard": np.ascontiguousarray(feats[rows]).astype(
                ml_dtypes.bfloat16),
            "a_lhsT": alt.astype(ml_dtypes.bfloat16),
            "wconv": wsb,
            "biases": bias,
        })
    return in_maps


def run(inputs, trace=False):
    in_maps = make_in_maps(**inputs)
    nc = _build_module(NPC, V, NCORES)
    res = bass_utils.run_bass_kernel_spmd(
        nc, in_maps, core_ids=list(range(NCORES)), trace=trace,
    )
    out = np.concatenate(
        [res.results[i]["out"] for i in range(NCORES)], axis=0
    ).reshape(V, C, H, H)
    return out, res


def kernel(**inputs):
    out, _ = run(inputs, trace=False)
    return out
